# revision 37
# baseline (speedup 1.0000x reference)
"""Trainium2 Bass kernel for nn_Actor (GNN message passing, B=65536, N=49).

Strategy
--------
The graph is fixed per call (edge_index + all weights are tiny inputs), so on
the host we fold the mean-aggregation matrix A (49x49) and the three SAGEConv
layers into dense matrices (float64 fold, exact). sage3 has no activation so
it folds into fc1. The network becomes a per-row MLP:

    [49] -> M1 [49,294] -> relu -> M2 [294,294] -> relu
         -> M3 [294,512] -> relu -> W4 [512,512] -> relu
         -> W5 [512,3] -> tanh -> *action_scale + action_bias

(294 = 6 feats x 49 nodes, feature-major order p = k*49 + n.)

Device side (8 NeuronCores, pure data parallel over batch), per core:

* Transposed layout: features on SBUF partitions, batch on the matmul free
  dimension, so every layer is a stationary-weight matmul streaming the
  batch, and bias+activation fuse into the PSUM->SBUF eviction on
  ScalarE/VectorE (alternated).
* fp16 compute (1 PE cycle/row like bf16, but 10 mantissa bits).
* Super-tiles: SUB=4 batch-tiles of BT=512 share each stationary weight,
  loop order m -> k -> s, giving evictions a full m-group of slack.
* Row-group packing: K<=64 chunks (L1's K=49, the 38-row tails of 294) are
  placed at PE row offsets 0/64 alternating by sub-tile parity via
  tile_position, so consecutive sub-tile matmuls run concurrently.
* Row-blocked x layout (partition p owns 64 consecutive rows) makes the x
  load 4 fat contiguous DMAs; x is cast to fp16 and transposed on the
  TensorEngine (identity matmul).
* The output is written transposed ([3, 8192] per core, batch order
  permuted by the tiling); gather() undoes the permutation on the host
  while unsharding.

Measured on 8 axon TRN2 cores: ~169 us NEFF exec, rel err 9.4e-4 vs the
fp32 reference. TensorE runs at ~216 ns per 512-col matmul (its roofline);
remaining time is ~11 us startup (Tile preamble + first DMAs) and ~7 us
drain tail.
"""

import os
import sys

for _p in ("/opt/trn_rl_repo", "/root/.axon_site/_ro/trn_rl_repo"):
    if os.path.isdir(_p) and _p not in sys.path:
        sys.path.append(_p)

import numpy as np

N = 49
B = 65536
N_CORES = 8
B_CORE = B // N_CORES          # 8192
BT = 512                       # batch tile (matmul free dim)
N_TILES = B_CORE // BT         # 16

COMPUTE = os.environ.get("K_COMPUTE", "fp16")   # "f32r" | "bf16" | "fp16"
BT = int(os.environ.get("K_BT", BT))
SUB = int(os.environ.get("K_SUB", "4"))  # batch-tiles per super-tile
N_TILES = B_CORE // BT

# Layer dims: K -> M
DIMS = [(49, 294), (294, 294), (294, 512), (512, 512), (512, 3)]


def _chunks(dim):
    out = []
    s = 0
    while s < dim:
        c = min(128, dim - s)
        out.append((s, c))
        s += c
    return out


# ----------------------------------------------------------------- host fold

def fold_weights(inputs):
    f8 = np.float64
    ei = np.asarray(inputs['edge_index'])
    src, dst = ei[0].astype(np.int64), ei[1].astype(np.int64)
    C = np.zeros((N, N), f8)
    np.add.at(C, (dst, src), 1.0)
    cnt = C.sum(axis=1)
    A = C / np.clip(cnt, 1.0, None)[:, None]
    I = np.eye(N, dtype=f8)

    W1l = np.asarray(inputs['W1l'], f8); W1r = np.asarray(inputs['W1r'], f8)
    b1 = np.asarray(inputs['b1'], f8)
    W2l = np.asarray(inputs['W2l'], f8); W2r = np.asarray(inputs['W2r'], f8)
    b2 = np.asarray(inputs['b2'], f8)
    W3l = np.asarray(inputs['W3l'], f8); W3r = np.asarray(inputs['W3r'], f8)
    b3 = np.asarray(inputs['b3'], f8)
    fc1_w = np.asarray(inputs['fc1_w'], f8); fc1_b = np.asarray(inputs['fc1_b'], f8)

    M1 = np.zeros((N, 6 * N), f8)
    for k in range(6):
        M1[:, k * N:(k + 1) * N] = A.T * W1l[0, k] + I * W1r[0, k]
    B1 = np.repeat(b1, N)

    M2 = np.zeros((6 * N, 6 * N), f8)
    for k in range(6):
        for k2 in range(6):
            M2[k * N:(k + 1) * N, k2 * N:(k2 + 1) * N] = \
                A.T * W2l[k, k2] + I * W2r[k, k2]
    B2 = np.repeat(b2, N)

    F = fc1_w.reshape(N, 12, 512)
    T1 = np.einsum('nm,kf,nfc->kmc', A, W3l, F, optimize=True)
    T2 = np.einsum('kf,mfc->kmc', W3r, F, optimize=True)
    M3 = (T1 + T2).reshape(6 * N, 512)
    B3 = np.einsum('f,nfc->c', b3, F, optimize=True) + fc1_b

    return dict(
        w=[M1, M2, M3, np.asarray(inputs['fc2_w'], f8), np.asarray(inputs['mu_w'], f8)],
        b=[B1, B2, B3, np.asarray(inputs['fc2_b'], f8), np.asarray(inputs['mu_b'], f8)],
        asc=np.asarray(inputs['action_scale'], f8),
        abi=np.asarray(inputs['action_bias'], f8),
    )


# -------------------------------------------------------------- bass builder

def build_nc(use_bias, use_scale, compute=COMPUTE):
    import concourse.bass as bass
    import concourse.mybir as mybir
    import concourse.tile as tile
    from concourse import bacc
    from concourse.masks import make_identity

    f32 = mybir.dt.float32
    if compute == "f32r":
        store_dt = mybir.dt.float32r   # maps to np.float32 on the host side
    elif compute == "fp16":
        store_dt = mybir.dt.float16
    else:
        store_dt = mybir.dt.bfloat16

    nc = bacc.Bacc("TRN2", target_bir_lowering=False, debug=False,
                   num_devices=N_CORES)

    x_dt = f32 if compute == "f32r" else store_dt
    x_d = nc.declare_dram_parameter("x", [B_CORE, N], x_dt, isOutput=False)
    w_d = [nc.declare_dram_parameter(f"w{i}", list(DIMS[i]), store_dt,
                                     isOutput=False) for i in range(5)]
    b_d = None
    if use_bias:
        b_d = [nc.declare_dram_parameter(f"b{i}", [DIMS[i][1]], f32,
                                         isOutput=False) for i in range(5)]
    s_d = None
    if use_scale:
        s_d = [nc.declare_dram_parameter("asc", [3], f32, isOutput=False),
               nc.declare_dram_parameter("abi", [3], f32, isOutput=False)]
    out_d = nc.declare_dram_parameter("out", [3, B_CORE], f32, isOutput=True)
    id_d = nc.declare_dram_parameter("ident", [128, 128],
                                     f32 if compute == "f32r" else store_dt,
                                     isOutput=False)

    Relu = mybir.ActivationFunctionType.Relu
    Tanh = mybir.ActivationFunctionType.Tanh

    with tile.TileContext(nc) as tc:
        with (
            tc.tile_pool(name="const", bufs=1) as const,
            tc.tile_pool(name="xin", bufs=8) as xin,
            tc.tile_pool(name="acts", bufs=44) as acts,
            tc.tile_pool(name="outs", bufs=8) as outs,
            tc.tile_pool(name="psum", bufs=(8 if BT <= 512 else 4), space="PSUM") as psum,
        ):
            ident = const.tile([128, 128],
                               f32 if compute == "f32r" else store_dt,
                               tag="ident")

            # Row-blocked batch layout: partition p owns rows p*64 .. p*64+63
            # of the core shard, so DRAM transfers are long contiguous runs
            # per partition. Column j of batch-tile t maps to batch
            # p*64 + (4t + j//128), p = j % 128.
            R = B_CORE // 128                       # 64 row-slots/partition
            RS = R // (N_TILES // SUB)              # 16 slots per super-tile
            x_rb = x_d.rearrange("(p r) f -> p r f", p=128)

            # DMA issue order matters: the first super-tile's x, then the
            # early-layer weights, then the rest of x, then the big weights.
            def _x_load(st, split=False):
                # for the first super-tile, split the DMA so the first
                # transpose can start as soon as the first chunk lands
                x_raw = xin.tile([128, RS, N], x_dt, name="x_raw",
                                 tag="x_raw")
                if split:
                    js = RS // SUB
                    for s in range(SUB):
                        nc.sync.dma_start(
                            out=x_raw[:, s * js:(s + 1) * js, :],
                            in_=x_rb[:, st * RS + s * js:
                                     st * RS + (s + 1) * js, :])
                else:
                    nc.sync.dma_start(
                        out=x_raw, in_=x_rb[:, st * RS:(st + 1) * RS, :])
                return x_raw

            def _w_load(i):
                K, M = DIMS[i]
                tiles = []
                for (ks, kc) in _chunks(K):
                    if kc <= 64:
                        # packed chunk: duplicate weights at row offsets 0/64
                        t = const.tile([128, M], store_dt, name=f"w{i}_{ks}",
                                       tag=f"w{i}_{ks}")
                        nc.sync.dma_start(out=t[0:kc, :],
                                          in_=w_d[i][ks:ks + kc, :])
                        nc.sync.dma_start(out=t[64:64 + kc, :],
                                          in_=w_d[i][ks:ks + kc, :])
                    else:
                        t = const.tile([kc, M], store_dt, name=f"w{i}_{ks}",
                                       tag=f"w{i}_{ks}")
                        nc.sync.dma_start(out=t, in_=w_d[i][ks:ks + kc, :])
                    tiles.append(t)
                return tiles

            x_raws = {0: _x_load(0)}
            nc.sync.dma_start(out=ident, in_=id_d[:, :])
            w_sb = [None] * 5
            for i in (0, 1, 2):
                w_sb[i] = _w_load(i)
            for st in range(1, N_TILES // SUB):
                x_raws[st] = _x_load(st)
            for i in (3, 4):
                w_sb[i] = _w_load(i)

            b_sb = None
            if use_bias:
                b_sb = []
                for i, (K, M) in enumerate(DIMS):
                    tiles = []
                    for (ms, mc) in _chunks(M):
                        t = const.tile([mc, 1], f32, tag=f"b{i}_{ms}")
                        nc.sync.dma_start(
                            out=t,
                            in_=b_d[i][ms:ms + mc].rearrange("(p o) -> p o", o=1))
                        tiles.append(t)
                    b_sb.append(tiles)

            if use_scale:
                asc_sb = const.tile([3, 1], f32, tag="asc")
                abi_sb = const.tile([3, 1], f32, tag="abi")
                nc.sync.dma_start(out=asc_sb,
                                  in_=s_d[0][:].rearrange("(p o) -> p o", o=1))
                nc.sync.dma_start(out=abi_sb,
                                  in_=s_d[1][:].rearrange("(p o) -> p o", o=1))

            # eviction engine round-robin: ScalarE (fused act) / VectorE
            rr = [0]

            def evict(h, ps, layer, mi):
                if layer == 4:  # tanh layer, must be ScalarE
                    bias = b_sb[4][0] if use_bias else 0.0
                    nc.scalar.activation(out=h, in_=ps, func=Tanh, bias=bias)
                    return
                use_scalar = (rr[0] % 2 == 0)
                rr[0] += 1
                if use_scalar:
                    bias = b_sb[layer][mi] if use_bias else 0.0
                    nc.scalar.activation(out=h, in_=ps, func=Relu, bias=bias)
                elif use_bias:
                    nc.vector.tensor_scalar(
                        out=h, in0=ps, scalar1=b_sb[layer][mi], scalar2=0.0,
                        op0=mybir.AluOpType.add, op1=mybir.AluOpType.max)
                else:
                    nc.vector.tensor_scalar_max(h, ps, 0.0)

            # Super-tile loop: SUB batch-tiles share each stationary weight
            # load (loop order m -> k -> s keeps the same lhsT for SUB
            # consecutive matmuls and gives PSUM evictions a full m-group of
            # slack before their results are consumed).
            JS = BT // 128

            def _xprep(st):
                x_src = x_raws[st]
                xT = []
                for s in range(SUB):
                    base = 0 if s % 2 == 0 else 64
                    ps_x = psum.tile([128, BT], store_dt, name="ps_x",
                                     tag="ps")
                    for j in range(JS):
                        nc.tensor.transpose(
                            ps_x[base:base + N, j * 128:(j + 1) * 128],
                            x_src[:, s * JS + j, :],
                            ident, tile_position=(0, base))
                    xt_s = acts.tile([128, BT], store_dt, name="xt",
                                     tag="xt")
                    nc.vector.tensor_copy(xt_s[base:base + N, :],
                                          ps_x[base:base + N, :])
                    xT.append((xt_s, base))
                return xT

            n_st = N_TILES // SUB
            for st in range(n_st):
                ts0 = st * SUB
                xT = _xprep(st)

                # ---- MLP chain.  cur[k][s] = (tile, base): activations for
                # K-chunk k of sub-tile s live at partition offset `base`.
                # Chunks narrower than 64 rows alternate base 0/64 by s
                # parity so consecutive sub-tile matmuls occupy disjoint
                # PE row-groups and run concurrently (tile_position).
                cur = [xT]
                ps5 = None
                t5 = outs.tile([3, SUB * BT], f32, tag="t5")

                def _l5_mms(ki, hs, ps5_):
                    # L5 (512->3) k-group ki: accumulate into one shared
                    # PSUM bank, each sub-tile at col offset 32*s
                    for s in range(SUB):
                        nc.tensor.matmul(
                            ps5_[32 * s:32 * s + 3, :],
                            w_sb[4][ki][:, 0:3], hs[s][0][0:128, :],
                            start=(ki == 0), stop=(ki == 3),
                            tile_position=(0, 32 * s))

                for layer, (K, M) in enumerate(DIMS[:4]):
                    kchunks = _chunks(K)
                    mchunks = _chunks(M)
                    nxt = []
                    if layer == 3:
                        ps5 = psum.tile([99, BT], f32, name="ps5", tag="ps")
                    for mi, (ms, mc) in enumerate(mchunks):
                        # where this output chunk must live for the consumer
                        packs_next = (layer < 4 and mc <= 64)
                        pss = [None] * SUB
                        obases = []
                        for s in range(SUB):
                            ob = 64 if (packs_next and s % 2 == 1) else 0
                            obases.append(ob)
                        for ki, (ks, kc) in enumerate(kchunks):
                            for s in range(SUB):
                                if pss[s] is None:
                                    # allocate at first use: spreads the
                                    # slot-wait out over the k0 sweep
                                    pss[s] = psum.tile([128, BT], f32,
                                                       name="ps", tag="ps")
                                h_t, ib = cur[ki][s]
                                nc.tensor.matmul(
                                    pss[s][obases[s]:obases[s] + mc, :],
                                    w_sb[layer][ki][ib:ib + kc, ms:ms + mc],
                                    h_t[ib:ib + kc, :],
                                    start=(ki == 0),
                                    stop=(ki == len(kchunks) - 1),
                                    tile_position=(ib, obases[s]))
                        if layer == 3 and mi >= 1:
                            _l5_mms(mi - 1, nxt[mi - 1], ps5)
                        hs = []
                        for s in range(SUB):
                            ob = obases[s]
                            h = acts.tile([128, BT], store_dt, name="h",
                                          tag="h")
                            evict(h[ob:ob + mc, :],
                                  pss[s][ob:ob + mc, :], layer, mi)
                            hs.append((h, ob))
                        nxt.append(hs)
                    cur = nxt

                _l5_mms(3, cur[3], ps5)
                for s in range(SUB):
                    evict(t5[:, s * BT:(s + 1) * BT],
                          ps5[32 * s:32 * s + 3, :], 4, 0)
                if use_scale:
                    t5s = outs.tile([3, SUB * BT], f32, tag="t5s")
                    nc.vector.tensor_scalar(
                        out=t5s, in0=t5, scalar1=asc_sb, scalar2=abi_sb,
                        op0=mybir.AluOpType.mult, op1=mybir.AluOpType.add)
                    t5 = t5s
                # one store per super-tile, transposed+permuted; the host
                # undoes the column permutation during unshard
                nc.sync.dma_start(
                    out=out_d[:, ts0 * BT:(ts0 + SUB) * BT], in_=t5)

    nc.compile()
    return nc


_CACHE = {}


def _np_store_dtype():
    if COMPUTE == "f32r":
        return np.float32
    if COMPUTE == "fp16":
        return np.float16
    import ml_dtypes
    return ml_dtypes.bfloat16


def _get_nc(use_bias, use_scale):
    key = (use_bias, use_scale, COMPUTE, BT, SUB)
    if key not in _CACHE:
        _CACHE[key] = build_nc(use_bias, use_scale)
    return _CACHE[key]


def prepare(inputs):
    """Fold weights and build per-core input maps. Returns (nc, in_maps)."""
    fw = fold_weights(inputs)
    use_bias = any(np.any(bi != 0.0) for bi in fw['b'])
    use_scale = bool(np.any(fw['asc'] != 1.0) or np.any(fw['abi'] != 0.0))
    nc = _get_nc(use_bias, use_scale)

    sdt = _np_store_dtype()
    wcast = [np.ascontiguousarray(w.astype(sdt)) for w in fw['w']]
    xdt = np.float32 if COMPUTE == "f32r" else sdt
    x = np.ascontiguousarray(np.asarray(inputs['x'], np.float32)
                             .reshape(B, N).astype(xdt))

    ident_np = np.eye(128, dtype=xdt)
    in_maps = []
    for c in range(N_CORES):
        m = {"x": np.ascontiguousarray(x[c * B_CORE:(c + 1) * B_CORE]),
             "ident": ident_np}
        for i in range(5):
            m[f"w{i}"] = wcast[i]
        if use_bias:
            for i in range(5):
                m[f"b{i}"] = fw['b'][i].astype(np.float32)
        if use_scale:
            m["asc"] = fw['asc'].astype(np.float32)
            m["abi"] = fw['abi'].astype(np.float32)
        in_maps.append(m)
    return nc, in_maps


def gather(results):
    """Unshard: per-core out is [3, B_CORE] with column (t, j_hi, p) ->
    batch p*R + t*J + j_hi. Undo the permutation and stack cores."""
    J = BT // 128
    parts = []
    for c in range(N_CORES):
        o = np.asarray(results[c]["out"])          # [3, B_CORE]
        o = o.reshape(3, N_TILES, J, 128).transpose(3, 1, 2, 0)
        parts.append(o.reshape(B_CORE, 3))
    return np.ascontiguousarray(np.concatenate(parts, axis=0)
                                .astype(np.float32))


def kernel(**inputs):
    from concourse.bass_utils import run_bass_kernel_spmd

    nc, in_maps = prepare(inputs)
    res = run_bass_kernel_spmd(nc, in_maps, core_ids=list(range(N_CORES)))
    return gather(res.results)


# revision 38
# speedup vs baseline: 1.1706x; 1.1706x over previous
"""Trainium2 Bass kernel for nn_Actor (GNN message passing, B=65536, N=49).

Strategy
--------
The graph is fixed per call (edge_index + all weights are tiny inputs), so on
the host we fold the mean-aggregation matrix A (49x49) and the three SAGEConv
layers into dense matrices (float64 fold, exact). sage3 has no activation so
it folds into fc1. The network becomes a per-row MLP:

    [49] -> M1 [49,294] -> relu -> M2 [294,294] -> relu
         -> M3 [294,512] -> relu -> W4 [512,512] -> relu
         -> W5 [512,3] -> tanh -> *action_scale + action_bias

(294 = 6 feats x 49 nodes, feature-major order p = k*49 + n.)

Device side (8 NeuronCores, pure data parallel over batch), per core:

* Transposed layout: features on SBUF partitions, batch on the matmul free
  dimension, so every layer is a stationary-weight matmul streaming the
  batch, and bias+activation fuse into the PSUM->SBUF eviction on
  ScalarE/VectorE (alternated).
* fp16 compute (1 PE cycle/row like bf16, but 10 mantissa bits).
* Super-tiles: SUB=4 batch-tiles of BT=512 share each stationary weight,
  loop order m -> k -> s, giving evictions a full m-group of slack.
* Row-group packing: K<=64 chunks (L1's K=49, the 38-row tails of 294) are
  placed at PE row offsets 0/64 alternating by sub-tile parity via
  tile_position, so consecutive sub-tile matmuls run concurrently.
* Row-blocked x layout (partition p owns 64 consecutive rows) makes the x
  load 4 fat contiguous DMAs; x is cast to fp16 and transposed on the
  TensorEngine (identity matmul).
* The output is written transposed ([3, 8192] per core, batch order
  permuted by the tiling); gather() undoes the permutation on the host
  while unsharding.

Measured on 8 axon TRN2 cores: ~169 us NEFF exec, rel err 9.4e-4 vs the
fp32 reference. TensorE runs at ~216 ns per 512-col matmul (its roofline);
remaining time is ~11 us startup (Tile preamble + first DMAs) and ~7 us
drain tail.
"""

import os
import sys

for _p in ("/opt/trn_rl_repo", "/root/.axon_site/_ro/trn_rl_repo"):
    if os.path.isdir(_p) and _p not in sys.path:
        sys.path.append(_p)

import numpy as np

N = 49
B = 65536
N_CORES = 8
B_CORE = B // N_CORES          # 8192
BT = 512                       # batch tile (matmul free dim)
N_TILES = B_CORE // BT         # 16

COMPUTE = os.environ.get("K_COMPUTE", "fp16")   # "f32r" | "bf16" | "fp16"
BT = int(os.environ.get("K_BT", BT))
SUB = int(os.environ.get("K_SUB", "4"))  # batch-tiles per super-tile
N_TILES = B_CORE // BT

# Layer dims: K -> M
DIMS = [(49, 294), (294, 294), (294, 512), (512, 512), (512, 3)]


def _chunks(dim):
    out = []
    s = 0
    while s < dim:
        c = min(128, dim - s)
        out.append((s, c))
        s += c
    return out


# ----------------------------------------------------------------- host fold

def fold_weights(inputs):
    f8 = np.float64
    ei = np.asarray(inputs['edge_index'])
    src, dst = ei[0].astype(np.int64), ei[1].astype(np.int64)
    C = np.zeros((N, N), f8)
    np.add.at(C, (dst, src), 1.0)
    cnt = C.sum(axis=1)
    A = C / np.clip(cnt, 1.0, None)[:, None]
    I = np.eye(N, dtype=f8)

    W1l = np.asarray(inputs['W1l'], f8); W1r = np.asarray(inputs['W1r'], f8)
    b1 = np.asarray(inputs['b1'], f8)
    W2l = np.asarray(inputs['W2l'], f8); W2r = np.asarray(inputs['W2r'], f8)
    b2 = np.asarray(inputs['b2'], f8)
    W3l = np.asarray(inputs['W3l'], f8); W3r = np.asarray(inputs['W3r'], f8)
    b3 = np.asarray(inputs['b3'], f8)
    fc1_w = np.asarray(inputs['fc1_w'], f8); fc1_b = np.asarray(inputs['fc1_b'], f8)

    M1 = np.zeros((N, 6 * N), f8)
    for k in range(6):
        M1[:, k * N:(k + 1) * N] = A.T * W1l[0, k] + I * W1r[0, k]
    B1 = np.repeat(b1, N)

    M2 = np.zeros((6 * N, 6 * N), f8)
    for k in range(6):
        for k2 in range(6):
            M2[k * N:(k + 1) * N, k2 * N:(k2 + 1) * N] = \
                A.T * W2l[k, k2] + I * W2r[k, k2]
    B2 = np.repeat(b2, N)

    F = fc1_w.reshape(N, 12, 512)
    T1 = np.einsum('nm,kf,nfc->kmc', A, W3l, F, optimize=True)
    T2 = np.einsum('kf,mfc->kmc', W3r, F, optimize=True)
    M3 = (T1 + T2).reshape(6 * N, 512)
    B3 = np.einsum('f,nfc->c', b3, F, optimize=True) + fc1_b

    return dict(
        w=[M1, M2, M3, np.asarray(inputs['fc2_w'], f8), np.asarray(inputs['mu_w'], f8)],
        b=[B1, B2, B3, np.asarray(inputs['fc2_b'], f8), np.asarray(inputs['mu_b'], f8)],
        asc=np.asarray(inputs['action_scale'], f8),
        abi=np.asarray(inputs['action_bias'], f8),
    )


# -------------------------------------------------------------- bass builder

def build_nc(use_bias, use_scale, compute=COMPUTE):
    import concourse.bass as bass
    import concourse.mybir as mybir
    import concourse.tile as tile
    from concourse import bacc
    from concourse.masks import make_identity

    f32 = mybir.dt.float32
    if compute == "f32r":
        store_dt = mybir.dt.float32r   # maps to np.float32 on the host side
    elif compute == "fp16":
        store_dt = mybir.dt.float16
    else:
        store_dt = mybir.dt.bfloat16

    nc = bacc.Bacc("TRN2", target_bir_lowering=False, debug=False,
                   num_devices=N_CORES)

    x_dt = f32 if compute == "f32r" else store_dt
    x_d = nc.declare_dram_parameter("x", [B_CORE, N], x_dt, isOutput=False)
    w_d = [nc.declare_dram_parameter(f"w{i}", list(DIMS[i]), store_dt,
                                     isOutput=False) for i in range(5)]
    b_d = None
    if use_bias:
        b_d = [nc.declare_dram_parameter(f"b{i}", [DIMS[i][1]], f32,
                                         isOutput=False) for i in range(5)]
    s_d = None
    if use_scale:
        s_d = [nc.declare_dram_parameter("asc", [3], f32, isOutput=False),
               nc.declare_dram_parameter("abi", [3], f32, isOutput=False)]
    out_d = nc.declare_dram_parameter("out", [3, B_CORE], f32, isOutput=True)
    id_d = nc.declare_dram_parameter("ident", [128, 128],
                                     f32 if compute == "f32r" else store_dt,
                                     isOutput=False)

    Relu = mybir.ActivationFunctionType.Relu
    Tanh = mybir.ActivationFunctionType.Tanh

    with tile.TileContext(nc) as tc:
        with (
            tc.tile_pool(name="const", bufs=1) as const,
            tc.tile_pool(name="xin", bufs=8) as xin,
            tc.tile_pool(name="acts", bufs=44) as acts,
            tc.tile_pool(name="outs", bufs=8) as outs,
            tc.tile_pool(name="psum", bufs=(8 if BT <= 512 else 4), space="PSUM") as psum,
        ):
            ident = const.tile([128, 128],
                               f32 if compute == "f32r" else store_dt,
                               tag="ident")

            # Row-blocked batch layout: partition p owns rows p*64 .. p*64+63
            # of the core shard, so DRAM transfers are long contiguous runs
            # per partition. Column j of batch-tile t maps to batch
            # p*64 + (4t + j//128), p = j % 128.
            R = B_CORE // 128                       # 64 row-slots/partition
            RS = R // (N_TILES // SUB)              # 16 slots per super-tile
            x_rb = x_d.rearrange("(p r) f -> p r f", p=128)

            # DMA issue order matters: the first super-tile's x, then the
            # early-layer weights, then the rest of x, then the big weights.
            def _x_load(st, split=False):
                # for the first super-tile, split the DMA so the first
                # transpose can start as soon as the first chunk lands
                x_raw = xin.tile([128, RS, N], x_dt, name="x_raw",
                                 tag="x_raw")
                if split:
                    js = RS // SUB
                    for s in range(SUB):
                        nc.sync.dma_start(
                            out=x_raw[:, s * js:(s + 1) * js, :],
                            in_=x_rb[:, st * RS + s * js:
                                     st * RS + (s + 1) * js, :])
                else:
                    nc.sync.dma_start(
                        out=x_raw, in_=x_rb[:, st * RS:(st + 1) * RS, :])
                return x_raw

            def _w_load(i):
                K, M = DIMS[i]
                tiles = []
                for (ks, kc) in _chunks(K):
                    if kc <= 64:
                        # packed chunk: duplicate weights at row offsets 0/64
                        t = const.tile([128, M], store_dt, name=f"w{i}_{ks}",
                                       tag=f"w{i}_{ks}")
                        nc.sync.dma_start(out=t[0:kc, :],
                                          in_=w_d[i][ks:ks + kc, :])
                        nc.sync.dma_start(out=t[64:64 + kc, :],
                                          in_=w_d[i][ks:ks + kc, :])
                    else:
                        t = const.tile([kc, M], store_dt, name=f"w{i}_{ks}",
                                       tag=f"w{i}_{ks}")
                        nc.sync.dma_start(out=t, in_=w_d[i][ks:ks + kc, :])
                    tiles.append(t)
                return tiles

            x_raws = {0: _x_load(0)}
            nc.sync.dma_start(out=ident, in_=id_d[:, :])
            w_sb = [None] * 5
            for i in (0, 1, 2):
                w_sb[i] = _w_load(i)
            for st in range(1, N_TILES // SUB):
                x_raws[st] = _x_load(st)
            for i in (3, 4):
                w_sb[i] = _w_load(i)

            b_sb = None
            if use_bias:
                b_sb = []
                for i, (K, M) in enumerate(DIMS):
                    tiles = []
                    for (ms, mc) in _chunks(M):
                        t = const.tile([mc, 1], f32, tag=f"b{i}_{ms}")
                        nc.sync.dma_start(
                            out=t,
                            in_=b_d[i][ms:ms + mc].rearrange("(p o) -> p o", o=1))
                        tiles.append(t)
                    b_sb.append(tiles)

            if use_scale:
                asc_sb = const.tile([3, 1], f32, tag="asc")
                abi_sb = const.tile([3, 1], f32, tag="abi")
                nc.sync.dma_start(out=asc_sb,
                                  in_=s_d[0][:].rearrange("(p o) -> p o", o=1))
                nc.sync.dma_start(out=abi_sb,
                                  in_=s_d[1][:].rearrange("(p o) -> p o", o=1))

            # eviction engine round-robin: ScalarE (fused act) / VectorE
            rr = [0]

            def evict(h, ps, layer, mi):
                if layer == 4:  # tanh layer, must be ScalarE
                    bias = b_sb[4][0] if use_bias else 0.0
                    nc.scalar.activation(out=h, in_=ps, func=Tanh, bias=bias)
                    return
                use_scalar = (rr[0] % 2 == 0)
                rr[0] += 1
                if use_scalar:
                    bias = b_sb[layer][mi] if use_bias else 0.0
                    nc.scalar.activation(out=h, in_=ps, func=Relu, bias=bias)
                elif use_bias:
                    nc.vector.tensor_scalar(
                        out=h, in0=ps, scalar1=b_sb[layer][mi], scalar2=0.0,
                        op0=mybir.AluOpType.add, op1=mybir.AluOpType.max)
                else:
                    nc.vector.tensor_scalar_max(h, ps, 0.0)

            # Super-tile loop: SUB batch-tiles share each stationary weight
            # load (loop order m -> k -> s keeps the same lhsT for SUB
            # consecutive matmuls and gives PSUM evictions a full m-group of
            # slack before their results are consumed).
            JS = BT // 128

            def _xprep(st):
                x_src = x_raws[st]
                # transposes write out partitions 0-48 (even s) or 64-112
                # (odd s): interleave s so adjacent ops use disjoint PE
                # column groups and run concurrently
                psx = []
                for s in range(SUB):
                    psx.append(psum.tile([128, BT], store_dt, name="ps_x",
                                         tag="ps"))
                for j in range(JS):
                    for s in range(SUB):
                        base = 0 if s % 2 == 0 else 64
                        nc.tensor.transpose(
                            psx[s][base:base + N, j * 128:(j + 1) * 128],
                            x_src[:, s * JS + j, :],
                            ident, tile_position=(0, base))
                xT = []
                for s in range(SUB):
                    base = 0 if s % 2 == 0 else 64
                    xt_s = acts.tile([128, BT], store_dt, name="xt",
                                     tag="xt")
                    nc.vector.tensor_copy(xt_s[base:base + N, :],
                                          psx[s][base:base + N, :])
                    xT.append((xt_s, base))
                return xT

            n_st = N_TILES // SUB
            for st in range(n_st):
                ts0 = st * SUB
                xT = _xprep(st)

                # ---- MLP chain.  cur[k][s] = (tile, base): activations for
                # K-chunk k of sub-tile s live at partition offset `base`.
                # Chunks narrower than 64 rows alternate base 0/64 by s
                # parity so consecutive sub-tile matmuls occupy disjoint
                # PE row-groups and run concurrently (tile_position).
                cur = [xT]
                ps5 = None
                t5 = outs.tile([3, SUB * BT], f32, tag="t5")

                def _l5_mms(ki, hs, ps5_):
                    # L5 (512->3) k-group ki: accumulate into one shared
                    # PSUM bank, each sub-tile at col offset 32*s
                    for s in range(SUB):
                        nc.tensor.matmul(
                            ps5_[32 * s:32 * s + 3, :],
                            w_sb[4][ki][:, 0:3], hs[s][0][0:128, :],
                            start=(ki == 0), stop=(ki == 3),
                            tile_position=(0, 32 * s))

                for layer, (K, M) in enumerate(DIMS[:4]):
                    kchunks = _chunks(K)
                    mchunks = _chunks(M)
                    nxt = []
                    if layer == 3:
                        ps5 = psum.tile([99, BT], f32, name="ps5", tag="ps")
                    for mi, (ms, mc) in enumerate(mchunks):
                        # where this output chunk must live for the consumer
                        packs_next = (layer < 4 and mc <= 64)
                        pss = [None] * SUB
                        obases = []
                        for s in range(SUB):
                            ob = 64 if (packs_next and s % 2 == 1) else 0
                            obases.append(ob)
                        for ki, (ks, kc) in enumerate(kchunks):
                            for s in range(SUB):
                                if pss[s] is None:
                                    # allocate at first use: spreads the
                                    # slot-wait out over the k0 sweep
                                    pss[s] = psum.tile([128, BT], f32,
                                                       name="ps", tag="ps")
                                h_t, ib = cur[ki][s]
                                nc.tensor.matmul(
                                    pss[s][obases[s]:obases[s] + mc, :],
                                    w_sb[layer][ki][ib:ib + kc, ms:ms + mc],
                                    h_t[ib:ib + kc, :],
                                    start=(ki == 0),
                                    stop=(ki == len(kchunks) - 1),
                                    tile_position=(ib, obases[s]))
                        if layer == 3 and mi >= 1:
                            _l5_mms(mi - 1, nxt[mi - 1], ps5)
                        hs = []
                        for s in range(SUB):
                            ob = obases[s]
                            h = acts.tile([128, BT], store_dt, name="h",
                                          tag="h")
                            evict(h[ob:ob + mc, :],
                                  pss[s][ob:ob + mc, :], layer, mi)
                            hs.append((h, ob))
                        nxt.append(hs)
                    cur = nxt

                _l5_mms(3, cur[3], ps5)
                for s in range(SUB):
                    evict(t5[:, s * BT:(s + 1) * BT],
                          ps5[32 * s:32 * s + 3, :], 4, 0)
                if use_scale:
                    t5s = outs.tile([3, SUB * BT], f32, tag="t5s")
                    nc.vector.tensor_scalar(
                        out=t5s, in0=t5, scalar1=asc_sb, scalar2=abi_sb,
                        op0=mybir.AluOpType.mult, op1=mybir.AluOpType.add)
                    t5 = t5s
                # one store per super-tile, transposed+permuted; the host
                # undoes the column permutation during unshard
                nc.sync.dma_start(
                    out=out_d[:, ts0 * BT:(ts0 + SUB) * BT], in_=t5)

    nc.compile()
    return nc


_CACHE = {}


def _np_store_dtype():
    if COMPUTE == "f32r":
        return np.float32
    if COMPUTE == "fp16":
        return np.float16
    import ml_dtypes
    return ml_dtypes.bfloat16


def _get_nc(use_bias, use_scale):
    key = (use_bias, use_scale, COMPUTE, BT, SUB)
    if key not in _CACHE:
        _CACHE[key] = build_nc(use_bias, use_scale)
    return _CACHE[key]


def prepare(inputs):
    """Fold weights and build per-core input maps. Returns (nc, in_maps)."""
    fw = fold_weights(inputs)
    use_bias = any(np.any(bi != 0.0) for bi in fw['b'])
    use_scale = bool(np.any(fw['asc'] != 1.0) or np.any(fw['abi'] != 0.0))
    nc = _get_nc(use_bias, use_scale)

    sdt = _np_store_dtype()
    wcast = [np.ascontiguousarray(w.astype(sdt)) for w in fw['w']]
    xdt = np.float32 if COMPUTE == "f32r" else sdt
    x = np.ascontiguousarray(np.asarray(inputs['x'], np.float32)
                             .reshape(B, N).astype(xdt))

    ident_np = np.eye(128, dtype=xdt)
    in_maps = []
    for c in range(N_CORES):
        m = {"x": np.ascontiguousarray(x[c * B_CORE:(c + 1) * B_CORE]),
             "ident": ident_np}
        for i in range(5):
            m[f"w{i}"] = wcast[i]
        if use_bias:
            for i in range(5):
                m[f"b{i}"] = fw['b'][i].astype(np.float32)
        if use_scale:
            m["asc"] = fw['asc'].astype(np.float32)
            m["abi"] = fw['abi'].astype(np.float32)
        in_maps.append(m)
    return nc, in_maps


def gather(results):
    """Unshard: per-core out is [3, B_CORE] with column (t, j_hi, p) ->
    batch p*R + t*J + j_hi. Undo the permutation and stack cores."""
    J = BT // 128
    parts = []
    for c in range(N_CORES):
        o = np.asarray(results[c]["out"])          # [3, B_CORE]
        o = o.reshape(3, N_TILES, J, 128).transpose(3, 1, 2, 0)
        parts.append(o.reshape(B_CORE, 3))
    return np.ascontiguousarray(np.concatenate(parts, axis=0)
                                .astype(np.float32))


def kernel(**inputs):
    from concourse.bass_utils import run_bass_kernel_spmd

    nc, in_maps = prepare(inputs)
    res = run_bass_kernel_spmd(nc, in_maps, core_ids=list(range(N_CORES)))
    return gather(res.results)


# revision 39
# speedup vs baseline: 1.2139x; 1.0370x over previous
"""Trainium2 Bass kernel for nn_Actor (GNN message passing, B=65536, N=49).

Strategy
--------
The graph is fixed per call (edge_index + all weights are tiny inputs), so on
the host we fold the mean-aggregation matrix A (49x49) and the three SAGEConv
layers into dense matrices (float64 fold, exact). sage3 has no activation so
it folds into fc1. The network becomes a per-row MLP:

    [49] -> M1 [49,294] -> relu -> M2 [294,294] -> relu
         -> M3 [294,512] -> relu -> W4 [512,512] -> relu
         -> W5 [512,3] -> tanh -> *action_scale + action_bias

(294 = 6 feats x 49 nodes, feature-major order p = k*49 + n.)

Device side (8 NeuronCores, pure data parallel over batch), per core:

* Transposed layout: features on SBUF partitions, batch on the matmul free
  dimension, so every layer is a stationary-weight matmul streaming the
  batch, and bias+activation fuse into the PSUM->SBUF eviction on
  ScalarE/VectorE (alternated).
* fp16 compute (1 PE cycle/row like bf16, but 10 mantissa bits).
* Super-tiles: SUB=4 batch-tiles of BT=512 share each stationary weight,
  loop order m -> k -> s, giving evictions a full m-group of slack.
* Row-group packing: K<=64 chunks (L1's K=49, the 38-row tails of 294) are
  placed at PE row offsets 0/64 alternating by sub-tile parity via
  tile_position, so consecutive sub-tile matmuls run concurrently.
* Row-blocked x layout (partition p owns 64 consecutive rows) makes the x
  load 4 fat contiguous DMAs; x is cast to fp16 and transposed on the
  TensorEngine (identity matmul).
* The output is written transposed ([3, 8192] per core, batch order
  permuted by the tiling); gather() undoes the permutation on the host
  while unsharding.

Measured on 8 axon TRN2 cores: ~169 us NEFF exec, rel err 9.4e-4 vs the
fp32 reference. TensorE runs at ~216 ns per 512-col matmul (its roofline);
remaining time is ~11 us startup (Tile preamble + first DMAs) and ~7 us
drain tail.
"""

import os
import sys

for _p in ("/opt/trn_rl_repo", "/root/.axon_site/_ro/trn_rl_repo"):
    if os.path.isdir(_p) and _p not in sys.path:
        sys.path.append(_p)

import numpy as np

N = 49
B = 65536
N_CORES = 8
B_CORE = B // N_CORES          # 8192
BT = 512                       # batch tile (matmul free dim)
N_TILES = B_CORE // BT         # 16

COMPUTE = os.environ.get("K_COMPUTE", "fp16")   # "f32r" | "bf16" | "fp16"
BT = int(os.environ.get("K_BT", BT))
SUB = int(os.environ.get("K_SUB", "4"))  # batch-tiles per super-tile
N_TILES = B_CORE // BT

# Layer dims: K -> M
DIMS = [(49, 294), (294, 294), (294, 512), (512, 512), (512, 3)]


def _chunks(dim):
    out = []
    s = 0
    while s < dim:
        c = min(128, dim - s)
        out.append((s, c))
        s += c
    return out


# ----------------------------------------------------------------- host fold

def fold_weights(inputs):
    f8 = np.float64
    ei = np.asarray(inputs['edge_index'])
    src, dst = ei[0].astype(np.int64), ei[1].astype(np.int64)
    C = np.zeros((N, N), f8)
    np.add.at(C, (dst, src), 1.0)
    cnt = C.sum(axis=1)
    A = C / np.clip(cnt, 1.0, None)[:, None]
    I = np.eye(N, dtype=f8)

    W1l = np.asarray(inputs['W1l'], f8); W1r = np.asarray(inputs['W1r'], f8)
    b1 = np.asarray(inputs['b1'], f8)
    W2l = np.asarray(inputs['W2l'], f8); W2r = np.asarray(inputs['W2r'], f8)
    b2 = np.asarray(inputs['b2'], f8)
    W3l = np.asarray(inputs['W3l'], f8); W3r = np.asarray(inputs['W3r'], f8)
    b3 = np.asarray(inputs['b3'], f8)
    fc1_w = np.asarray(inputs['fc1_w'], f8); fc1_b = np.asarray(inputs['fc1_b'], f8)

    M1 = np.zeros((N, 6 * N), f8)
    for k in range(6):
        M1[:, k * N:(k + 1) * N] = A.T * W1l[0, k] + I * W1r[0, k]
    B1 = np.repeat(b1, N)

    M2 = np.zeros((6 * N, 6 * N), f8)
    for k in range(6):
        for k2 in range(6):
            M2[k * N:(k + 1) * N, k2 * N:(k2 + 1) * N] = \
                A.T * W2l[k, k2] + I * W2r[k, k2]
    B2 = np.repeat(b2, N)

    F = fc1_w.reshape(N, 12, 512)
    T1 = np.einsum('nm,kf,nfc->kmc', A, W3l, F, optimize=True)
    T2 = np.einsum('kf,mfc->kmc', W3r, F, optimize=True)
    M3 = (T1 + T2).reshape(6 * N, 512)
    B3 = np.einsum('f,nfc->c', b3, F, optimize=True) + fc1_b

    return dict(
        w=[M1, M2, M3, np.asarray(inputs['fc2_w'], f8), np.asarray(inputs['mu_w'], f8)],
        b=[B1, B2, B3, np.asarray(inputs['fc2_b'], f8), np.asarray(inputs['mu_b'], f8)],
        asc=np.asarray(inputs['action_scale'], f8),
        abi=np.asarray(inputs['action_bias'], f8),
    )


# -------------------------------------------------------------- bass builder

def build_nc(use_bias, use_scale, compute=COMPUTE):
    import concourse.bass as bass
    import concourse.mybir as mybir
    import concourse.tile as tile
    from concourse import bacc
    from concourse.masks import make_identity

    f32 = mybir.dt.float32
    if compute == "f32r":
        store_dt = mybir.dt.float32r   # maps to np.float32 on the host side
    elif compute == "fp16":
        store_dt = mybir.dt.float16
    else:
        store_dt = mybir.dt.bfloat16

    nc = bacc.Bacc("TRN2", target_bir_lowering=False, debug=False,
                   num_devices=N_CORES)

    x_dt = f32 if compute == "f32r" else store_dt
    x_d = nc.declare_dram_parameter("x", [B_CORE, N], x_dt, isOutput=False)
    w_d = [nc.declare_dram_parameter(f"w{i}", list(DIMS[i]), store_dt,
                                     isOutput=False) for i in range(5)]
    b_d = None
    if use_bias:
        b_d = [nc.declare_dram_parameter(f"b{i}", [DIMS[i][1]], f32,
                                         isOutput=False) for i in range(5)]
    s_d = None
    if use_scale:
        s_d = [nc.declare_dram_parameter("asc", [3], f32, isOutput=False),
               nc.declare_dram_parameter("abi", [3], f32, isOutput=False)]
    out_d = nc.declare_dram_parameter("out", [3, B_CORE], f32, isOutput=True)
    id_d = nc.declare_dram_parameter("ident", [128, 128],
                                     f32 if compute == "f32r" else store_dt,
                                     isOutput=False)

    Relu = mybir.ActivationFunctionType.Relu
    Tanh = mybir.ActivationFunctionType.Tanh

    with tile.TileContext(nc) as tc:
        with (
            tc.tile_pool(name="const", bufs=1) as const,
            tc.tile_pool(name="xin", bufs=8) as xin,
            tc.tile_pool(name="acts", bufs=44) as acts,
            tc.tile_pool(name="outs", bufs=8) as outs,
            tc.tile_pool(name="psum", bufs=(8 if BT <= 512 else 4), space="PSUM") as psum,
        ):
            ident = const.tile([128, 128],
                               f32 if compute == "f32r" else store_dt,
                               tag="ident")

            # Row-blocked batch layout: partition p owns rows p*64 .. p*64+63
            # of the core shard, so DRAM transfers are long contiguous runs
            # per partition. Column j of batch-tile t maps to batch
            # p*64 + (4t + j//128), p = j % 128.
            R = B_CORE // 128                       # 64 row-slots/partition
            RS = R // (N_TILES // SUB)              # 16 slots per super-tile
            x_rb = x_d.rearrange("(p r) f -> p r f", p=128)

            # DMA issue order matters: the first super-tile's x, then the
            # early-layer weights, then the rest of x, then the big weights.
            def _x_load(st, split=False):
                # for the first super-tile, split the DMA so the first
                # transpose can start as soon as the first chunk lands
                x_raw = xin.tile([128, RS, N], x_dt, name="x_raw",
                                 tag="x_raw")
                if split:
                    js = RS // SUB
                    for s in range(SUB):
                        nc.sync.dma_start(
                            out=x_raw[:, s * js:(s + 1) * js, :],
                            in_=x_rb[:, st * RS + s * js:
                                     st * RS + (s + 1) * js, :])
                else:
                    nc.sync.dma_start(
                        out=x_raw, in_=x_rb[:, st * RS:(st + 1) * RS, :])
                return x_raw

            def _w_load(i):
                K, M = DIMS[i]
                tiles = []
                for (ks, kc) in _chunks(K):
                    if kc <= 64:
                        # packed chunk: duplicate weights at row offsets 0/64
                        t = const.tile([128, M], store_dt, name=f"w{i}_{ks}",
                                       tag=f"w{i}_{ks}")
                        nc.sync.dma_start(out=t[0:kc, :],
                                          in_=w_d[i][ks:ks + kc, :])
                        nc.sync.dma_start(out=t[64:64 + kc, :],
                                          in_=w_d[i][ks:ks + kc, :])
                    else:
                        t = const.tile([kc, M], store_dt, name=f"w{i}_{ks}",
                                       tag=f"w{i}_{ks}")
                        nc.sync.dma_start(out=t, in_=w_d[i][ks:ks + kc, :])
                    tiles.append(t)
                return tiles

            x_raws = {0: _x_load(0)}
            nc.sync.dma_start(out=ident, in_=id_d[:, :])
            w_sb = [None] * 5
            for i in (0, 1, 2):
                w_sb[i] = _w_load(i)
            for st in range(1, N_TILES // SUB):
                x_raws[st] = _x_load(st)
            for i in (3, 4):
                w_sb[i] = _w_load(i)

            b_sb = None
            if use_bias:
                b_sb = []
                for i, (K, M) in enumerate(DIMS):
                    tiles = []
                    for (ms, mc) in _chunks(M):
                        t = const.tile([mc, 1], f32, tag=f"b{i}_{ms}")
                        nc.sync.dma_start(
                            out=t,
                            in_=b_d[i][ms:ms + mc].rearrange("(p o) -> p o", o=1))
                        tiles.append(t)
                    b_sb.append(tiles)

            if use_scale:
                asc_sb = const.tile([3, 1], f32, tag="asc")
                abi_sb = const.tile([3, 1], f32, tag="abi")
                nc.sync.dma_start(out=asc_sb,
                                  in_=s_d[0][:].rearrange("(p o) -> p o", o=1))
                nc.sync.dma_start(out=abi_sb,
                                  in_=s_d[1][:].rearrange("(p o) -> p o", o=1))

            # eviction engine round-robin: ScalarE (fused act) / VectorE
            rr = [0]

            def evict(h, ps, layer, mi):
                if layer == 4:  # tanh layer, must be ScalarE
                    bias = b_sb[4][0] if use_bias else 0.0
                    nc.scalar.activation(out=h, in_=ps, func=Tanh, bias=bias)
                    return
                use_scalar = (rr[0] % 2 == 0)
                rr[0] += 1
                if use_scalar:
                    bias = b_sb[layer][mi] if use_bias else 0.0
                    nc.scalar.activation(out=h, in_=ps, func=Relu, bias=bias)
                elif use_bias:
                    nc.vector.tensor_scalar(
                        out=h, in0=ps, scalar1=b_sb[layer][mi], scalar2=0.0,
                        op0=mybir.AluOpType.add, op1=mybir.AluOpType.max)
                else:
                    nc.vector.tensor_scalar_max(h, ps, 0.0)

            # Super-tile loop: SUB batch-tiles share each stationary weight
            # load (loop order m -> k -> s keeps the same lhsT for SUB
            # consecutive matmuls and gives PSUM evictions a full m-group of
            # slack before their results are consumed).
            JS = BT // 128

            def _xprep(st):
                x_src = x_raws[st]
                xT = []
                for s in range(SUB):
                    base = 0 if s % 2 == 0 else 64
                    ps_x = psum.tile([128, BT], store_dt, name="ps_x",
                                     tag="ps")
                    for j in range(JS):
                        nc.tensor.transpose(
                            ps_x[base:base + N, j * 128:(j + 1) * 128],
                            x_src[:, s * JS + j, :],
                            ident, tile_position=(0, base))
                    xt_s = acts.tile([128, BT], store_dt, name="xt",
                                     tag="xt")
                    nc.vector.tensor_copy(xt_s[base:base + N, :],
                                          ps_x[base:base + N, :])
                    xT.append((xt_s, base))
                return xT

            n_st = N_TILES // SUB
            for st in range(n_st):
                ts0 = st * SUB
                xT = _xprep(st)

                # ---- MLP chain.  cur[k][s] = (tile, base): activations for
                # K-chunk k of sub-tile s live at partition offset `base`.
                # Chunks narrower than 64 rows alternate base 0/64 by s
                # parity so consecutive sub-tile matmuls occupy disjoint
                # PE row-groups and run concurrently (tile_position).
                cur = [xT]
                ps5 = None
                t5 = outs.tile([3, SUB * BT], f32, tag="t5")

                def _l5_mms(ki, hs, ps5_):
                    # L5 (512->3) k-group ki: accumulate into one shared
                    # PSUM bank, each sub-tile at col offset 32*s
                    for s in range(SUB):
                        nc.tensor.matmul(
                            ps5_[32 * s:32 * s + 3, :],
                            w_sb[4][ki][:, 0:3], hs[s][0][0:128, :],
                            start=(ki == 0), stop=(ki == 3),
                            tile_position=(0, 32 * s))

                for layer, (K, M) in enumerate(DIMS[:4]):
                    kchunks = _chunks(K)
                    mchunks = _chunks(M)
                    nxt = []
                    if layer == 3:
                        ps5 = psum.tile([99, BT], f32, name="ps5", tag="ps")
                    for mi, (ms, mc) in enumerate(mchunks):
                        # where this output chunk must live for the consumer
                        packs_next = (layer < 4 and mc <= 64)
                        pss = [None] * SUB
                        obases = []
                        for s in range(SUB):
                            ob = 64 if (packs_next and s % 2 == 1) else 0
                            obases.append(ob)
                        for ki, (ks, kc) in enumerate(kchunks):
                            for s in range(SUB):
                                if pss[s] is None:
                                    # allocate at first use: spreads the
                                    # slot-wait out over the k0 sweep
                                    pss[s] = psum.tile([128, BT], f32,
                                                       name="ps", tag="ps")
                                h_t, ib = cur[ki][s]
                                nc.tensor.matmul(
                                    pss[s][obases[s]:obases[s] + mc, :],
                                    w_sb[layer][ki][ib:ib + kc, ms:ms + mc],
                                    h_t[ib:ib + kc, :],
                                    start=(ki == 0),
                                    stop=(ki == len(kchunks) - 1),
                                    tile_position=(ib, obases[s]))
                        if layer == 3 and mi >= 1:
                            _l5_mms(mi - 1, nxt[mi - 1], ps5)
                        hs = []
                        for s in range(SUB):
                            ob = obases[s]
                            h = acts.tile([128, BT], store_dt, name="h",
                                          tag="h")
                            evict(h[ob:ob + mc, :],
                                  pss[s][ob:ob + mc, :], layer, mi)
                            hs.append((h, ob))
                        nxt.append(hs)
                    cur = nxt

                _l5_mms(3, cur[3], ps5)
                for s in range(SUB):
                    evict(t5[:, s * BT:(s + 1) * BT],
                          ps5[32 * s:32 * s + 3, :], 4, 0)
                if use_scale:
                    t5s = outs.tile([3, SUB * BT], f32, tag="t5s")
                    nc.vector.tensor_scalar(
                        out=t5s, in0=t5, scalar1=asc_sb, scalar2=abi_sb,
                        op0=mybir.AluOpType.mult, op1=mybir.AluOpType.add)
                    t5 = t5s
                # one store per super-tile, transposed+permuted; the host
                # undoes the column permutation during unshard
                nc.sync.dma_start(
                    out=out_d[:, ts0 * BT:(ts0 + SUB) * BT], in_=t5)

    nc.compile()
    return nc


_CACHE = {}


def _np_store_dtype():
    if COMPUTE == "f32r":
        return np.float32
    if COMPUTE == "fp16":
        return np.float16
    import ml_dtypes
    return ml_dtypes.bfloat16


def _get_nc(use_bias, use_scale):
    key = (use_bias, use_scale, COMPUTE, BT, SUB)
    if key not in _CACHE:
        _CACHE[key] = build_nc(use_bias, use_scale)
    return _CACHE[key]


def prepare(inputs):
    """Fold weights and build per-core input maps. Returns (nc, in_maps)."""
    fw = fold_weights(inputs)
    use_bias = any(np.any(bi != 0.0) for bi in fw['b'])
    use_scale = bool(np.any(fw['asc'] != 1.0) or np.any(fw['abi'] != 0.0))
    nc = _get_nc(use_bias, use_scale)

    sdt = _np_store_dtype()
    wcast = [np.ascontiguousarray(w.astype(sdt)) for w in fw['w']]
    xdt = np.float32 if COMPUTE == "f32r" else sdt
    x = np.ascontiguousarray(np.asarray(inputs['x'], np.float32)
                             .reshape(B, N).astype(xdt))

    ident_np = np.eye(128, dtype=xdt)
    in_maps = []
    for c in range(N_CORES):
        m = {"x": np.ascontiguousarray(x[c * B_CORE:(c + 1) * B_CORE]),
             "ident": ident_np}
        for i in range(5):
            m[f"w{i}"] = wcast[i]
        if use_bias:
            for i in range(5):
                m[f"b{i}"] = fw['b'][i].astype(np.float32)
        if use_scale:
            m["asc"] = fw['asc'].astype(np.float32)
            m["abi"] = fw['abi'].astype(np.float32)
        in_maps.append(m)
    return nc, in_maps


def gather(results):
    """Unshard: per-core out is [3, B_CORE] with column (t, j_hi, p) ->
    batch p*R + t*J + j_hi. Undo the permutation and stack cores."""
    J = BT // 128
    parts = []
    for c in range(N_CORES):
        o = np.asarray(results[c]["out"])          # [3, B_CORE]
        o = o.reshape(3, N_TILES, J, 128).transpose(3, 1, 2, 0)
        parts.append(o.reshape(B_CORE, 3))
    return np.ascontiguousarray(np.concatenate(parts, axis=0)
                                .astype(np.float32))


def kernel(**inputs):
    from concourse.bass_utils import run_bass_kernel_spmd

    nc, in_maps = prepare(inputs)
    res = run_bass_kernel_spmd(nc, in_maps, core_ids=list(range(N_CORES)))
    return gather(res.results)


# revision 40
# speedup vs baseline: 1.2155x; 1.0014x over previous
"""Trainium2 Bass kernel for nn_Actor (GNN message passing, B=65536, N=49).

Strategy
--------
The graph is fixed per call (edge_index + all weights are tiny inputs), so on
the host we fold the mean-aggregation matrix A (49x49) and the three SAGEConv
layers into dense matrices (float64 fold, exact). sage3 has no activation so
it folds into fc1. The network becomes a per-row MLP:

    [49] -> M1 [49,294] -> relu -> M2 [294,294] -> relu
         -> M3 [294,512] -> relu -> W4 [512,512] -> relu
         -> W5 [512,3] -> tanh -> *action_scale + action_bias

(294 = 6 feats x 49 nodes, feature-major order p = k*49 + n.)

Device side (8 NeuronCores, pure data parallel over batch), per core:

* Transposed layout: features on SBUF partitions, batch on the matmul free
  dimension, so every layer is a stationary-weight matmul streaming the
  batch, and bias+activation fuse into the PSUM->SBUF eviction on
  ScalarE/VectorE (alternated).
* fp16 compute (1 PE cycle/row like bf16, but 10 mantissa bits).
* Super-tiles: SUB=4 batch-tiles of BT=512 share each stationary weight,
  loop order m -> k -> s, giving evictions a full m-group of slack.
* PE array packing via tile_position: K<=64 chunks (L1's K=49, the 38-row
  tails of 294) alternate row offsets 0/64 by sub-tile parity, and narrow
  outputs (38-col chunks, L5's M=3) land at distinct column groups, so
  consecutive sub-tile matmuls occupy disjoint PE cells and run
  concurrently (issue gap ~3ns).
* L5 (512->3) is software-pipelined into L4's m-loop, its four sub-tile
  accumulators sharing one PSUM bank at column offsets 32*s.
* Row-blocked x layout (partition p owns 64 consecutive rows) makes the x
  load 4 fat contiguous DMAs; x is cast to fp16 and transposed on the
  TensorEngine (identity matmul).
* The output is written transposed ([3, 8192] per core, batch order
  permuted by the tiling); gather() undoes the permutation on the host
  while unsharding.

Measured on 8 axon TRN2 cores: ~167-169 us NEFF exec at 2.4GHz (the chip
sometimes sits in P0 at 2.0GHz: ~200 us), rel err 9.4e-4 vs the fp32
reference. TensorE is ~150 us busy at its 216 ns/512-col-matmul roofline;
the rest is ~10 us startup (Tile preamble + first DMAs) and ~6 us tail
(serial tanh evictions + drain).
"""

import os
import sys

for _p in ("/opt/trn_rl_repo", "/root/.axon_site/_ro/trn_rl_repo"):
    if os.path.isdir(_p) and _p not in sys.path:
        sys.path.append(_p)

import numpy as np

N = 49
B = 65536
N_CORES = 8
B_CORE = B // N_CORES          # 8192
BT = 512                       # batch tile (matmul free dim)
N_TILES = B_CORE // BT         # 16

COMPUTE = os.environ.get("K_COMPUTE", "fp16")   # "f32r" | "bf16" | "fp16"
BT = int(os.environ.get("K_BT", BT))
SUB = int(os.environ.get("K_SUB", "4"))  # batch-tiles per super-tile
N_TILES = B_CORE // BT

# Layer dims: K -> M
DIMS = [(49, 294), (294, 294), (294, 512), (512, 512), (512, 3)]


def _chunks(dim):
    out = []
    s = 0
    while s < dim:
        c = min(128, dim - s)
        out.append((s, c))
        s += c
    return out


# ----------------------------------------------------------------- host fold

def fold_weights(inputs):
    f8 = np.float64
    ei = np.asarray(inputs['edge_index'])
    src, dst = ei[0].astype(np.int64), ei[1].astype(np.int64)
    C = np.zeros((N, N), f8)
    np.add.at(C, (dst, src), 1.0)
    cnt = C.sum(axis=1)
    A = C / np.clip(cnt, 1.0, None)[:, None]
    I = np.eye(N, dtype=f8)

    W1l = np.asarray(inputs['W1l'], f8); W1r = np.asarray(inputs['W1r'], f8)
    b1 = np.asarray(inputs['b1'], f8)
    W2l = np.asarray(inputs['W2l'], f8); W2r = np.asarray(inputs['W2r'], f8)
    b2 = np.asarray(inputs['b2'], f8)
    W3l = np.asarray(inputs['W3l'], f8); W3r = np.asarray(inputs['W3r'], f8)
    b3 = np.asarray(inputs['b3'], f8)
    fc1_w = np.asarray(inputs['fc1_w'], f8); fc1_b = np.asarray(inputs['fc1_b'], f8)

    M1 = np.zeros((N, 6 * N), f8)
    for k in range(6):
        M1[:, k * N:(k + 1) * N] = A.T * W1l[0, k] + I * W1r[0, k]
    B1 = np.repeat(b1, N)

    M2 = np.zeros((6 * N, 6 * N), f8)
    for k in range(6):
        for k2 in range(6):
            M2[k * N:(k + 1) * N, k2 * N:(k2 + 1) * N] = \
                A.T * W2l[k, k2] + I * W2r[k, k2]
    B2 = np.repeat(b2, N)

    F = fc1_w.reshape(N, 12, 512)
    T1 = np.einsum('nm,kf,nfc->kmc', A, W3l, F, optimize=True)
    T2 = np.einsum('kf,mfc->kmc', W3r, F, optimize=True)
    M3 = (T1 + T2).reshape(6 * N, 512)
    B3 = np.einsum('f,nfc->c', b3, F, optimize=True) + fc1_b

    return dict(
        w=[M1, M2, M3, np.asarray(inputs['fc2_w'], f8), np.asarray(inputs['mu_w'], f8)],
        b=[B1, B2, B3, np.asarray(inputs['fc2_b'], f8), np.asarray(inputs['mu_b'], f8)],
        asc=np.asarray(inputs['action_scale'], f8),
        abi=np.asarray(inputs['action_bias'], f8),
    )


# -------------------------------------------------------------- bass builder

def build_nc(use_bias, use_scale, compute=COMPUTE):
    import concourse.bass as bass
    import concourse.mybir as mybir
    import concourse.tile as tile
    from concourse import bacc
    from concourse.masks import make_identity

    f32 = mybir.dt.float32
    if compute == "f32r":
        store_dt = mybir.dt.float32r   # maps to np.float32 on the host side
    elif compute == "fp16":
        store_dt = mybir.dt.float16
    else:
        store_dt = mybir.dt.bfloat16

    nc = bacc.Bacc("TRN2", target_bir_lowering=False, debug=False,
                   num_devices=N_CORES)

    x_dt = f32 if compute == "f32r" else store_dt
    x_d = nc.declare_dram_parameter("x", [B_CORE, N], x_dt, isOutput=False)
    w_d = [nc.declare_dram_parameter(f"w{i}", list(DIMS[i]), store_dt,
                                     isOutput=False) for i in range(5)]
    b_d = None
    if use_bias:
        b_d = [nc.declare_dram_parameter(f"b{i}", [DIMS[i][1]], f32,
                                         isOutput=False) for i in range(5)]
    s_d = None
    if use_scale:
        s_d = [nc.declare_dram_parameter("asc", [3], f32, isOutput=False),
               nc.declare_dram_parameter("abi", [3], f32, isOutput=False)]
    out_d = nc.declare_dram_parameter("out", [3, B_CORE], f32, isOutput=True)
    id_d = nc.declare_dram_parameter("ident", [128, 128],
                                     f32 if compute == "f32r" else store_dt,
                                     isOutput=False)

    Relu = mybir.ActivationFunctionType.Relu
    Tanh = mybir.ActivationFunctionType.Tanh

    with tile.TileContext(nc) as tc:
        with (
            tc.tile_pool(name="const", bufs=1) as const,
            tc.tile_pool(name="xin", bufs=8) as xin,
            tc.tile_pool(name="acts", bufs=44) as acts,
            tc.tile_pool(name="outs", bufs=8) as outs,
            tc.tile_pool(name="psum", bufs=(8 if BT <= 512 else 4), space="PSUM") as psum,
        ):
            ident = const.tile([128, 128],
                               f32 if compute == "f32r" else store_dt,
                               tag="ident")

            # Row-blocked batch layout: partition p owns rows p*64 .. p*64+63
            # of the core shard, so DRAM transfers are long contiguous runs
            # per partition. Column j of batch-tile t maps to batch
            # p*64 + (4t + j//128), p = j % 128.
            R = B_CORE // 128                       # 64 row-slots/partition
            RS = R // (N_TILES // SUB)              # 16 slots per super-tile
            x_rb = x_d.rearrange("(p r) f -> p r f", p=128)

            # DMA issue order matters: the first super-tile's x, then the
            # early-layer weights, then the rest of x, then the big weights.
            def _x_load(st, split=False):
                # for the first super-tile, split the DMA so the first
                # transpose can start as soon as the first chunk lands
                x_raw = xin.tile([128, RS, N], x_dt, name="x_raw",
                                 tag="x_raw")
                if split:
                    js = RS // SUB
                    for s in range(SUB):
                        nc.sync.dma_start(
                            out=x_raw[:, s * js:(s + 1) * js, :],
                            in_=x_rb[:, st * RS + s * js:
                                     st * RS + (s + 1) * js, :])
                else:
                    nc.sync.dma_start(
                        out=x_raw, in_=x_rb[:, st * RS:(st + 1) * RS, :])
                return x_raw

            def _w_load(i):
                K, M = DIMS[i]
                tiles = []
                for (ks, kc) in _chunks(K):
                    if kc <= 64:
                        # packed chunk: duplicate weights at row offsets 0/64
                        t = const.tile([128, M], store_dt, name=f"w{i}_{ks}",
                                       tag=f"w{i}_{ks}")
                        nc.sync.dma_start(out=t[0:kc, :],
                                          in_=w_d[i][ks:ks + kc, :])
                        nc.sync.dma_start(out=t[64:64 + kc, :],
                                          in_=w_d[i][ks:ks + kc, :])
                    else:
                        t = const.tile([kc, M], store_dt, name=f"w{i}_{ks}",
                                       tag=f"w{i}_{ks}")
                        nc.sync.dma_start(out=t, in_=w_d[i][ks:ks + kc, :])
                    tiles.append(t)
                return tiles

            x_raws = {0: _x_load(0)}
            nc.sync.dma_start(out=ident, in_=id_d[:, :])
            w_sb = [None] * 5
            for i in (0, 1, 2):
                w_sb[i] = _w_load(i)
            for st in range(1, N_TILES // SUB):
                x_raws[st] = _x_load(st)
            for i in (3, 4):
                w_sb[i] = _w_load(i)

            b_sb = None
            if use_bias:
                b_sb = []
                for i, (K, M) in enumerate(DIMS):
                    tiles = []
                    for (ms, mc) in _chunks(M):
                        t = const.tile([mc, 1], f32, tag=f"b{i}_{ms}")
                        nc.sync.dma_start(
                            out=t,
                            in_=b_d[i][ms:ms + mc].rearrange("(p o) -> p o", o=1))
                        tiles.append(t)
                    b_sb.append(tiles)

            if use_scale:
                asc_sb = const.tile([3, 1], f32, tag="asc")
                abi_sb = const.tile([3, 1], f32, tag="abi")
                nc.sync.dma_start(out=asc_sb,
                                  in_=s_d[0][:].rearrange("(p o) -> p o", o=1))
                nc.sync.dma_start(out=abi_sb,
                                  in_=s_d[1][:].rearrange("(p o) -> p o", o=1))

            # eviction engine round-robin: ScalarE (fused act) / VectorE
            rr = [0]

            def evict(h, ps, layer, mi):
                if layer == 4:  # tanh layer, must be ScalarE
                    bias = b_sb[4][0] if use_bias else 0.0
                    nc.scalar.activation(out=h, in_=ps, func=Tanh, bias=bias)
                    return
                use_scalar = (rr[0] % 2 == 0)
                rr[0] += 1
                if use_scalar:
                    bias = b_sb[layer][mi] if use_bias else 0.0
                    nc.scalar.activation(out=h, in_=ps, func=Relu, bias=bias)
                elif use_bias:
                    nc.vector.tensor_scalar(
                        out=h, in0=ps, scalar1=b_sb[layer][mi], scalar2=0.0,
                        op0=mybir.AluOpType.add, op1=mybir.AluOpType.max)
                else:
                    nc.vector.tensor_scalar_max(h, ps, 0.0)

            # Super-tile loop: SUB batch-tiles share each stationary weight
            # load (loop order m -> k -> s keeps the same lhsT for SUB
            # consecutive matmuls and gives PSUM evictions a full m-group of
            # slack before their results are consumed).
            JS = BT // 128

            def _xprep(st):
                x_src = x_raws[st]
                xT = []
                for s in range(SUB):
                    base = 0 if s % 2 == 0 else 64
                    ps_x = psum.tile([128, BT], store_dt, name="ps_x",
                                     tag="ps")
                    for j in range(JS):
                        nc.tensor.transpose(
                            ps_x[base:base + N, j * 128:(j + 1) * 128],
                            x_src[:, s * JS + j, :],
                            ident, tile_position=(0, base))
                    xt_s = acts.tile([128, BT], store_dt, name="xt",
                                     tag="xt")
                    nc.vector.tensor_copy(xt_s[base:base + N, :],
                                          ps_x[base:base + N, :])
                    xT.append((xt_s, base))
                return xT

            n_st = N_TILES // SUB
            for st in range(n_st):
                ts0 = st * SUB
                xT = _xprep(st)

                # ---- MLP chain.  cur[k][s] = (tile, base): activations for
                # K-chunk k of sub-tile s live at partition offset `base`.
                # Chunks narrower than 64 rows alternate base 0/64 by s
                # parity so consecutive sub-tile matmuls occupy disjoint
                # PE row-groups and run concurrently (tile_position).
                cur = [xT]
                ps5 = None
                t5 = outs.tile([3, SUB * BT], f32, tag="t5")

                def _l5_mms(ki, hs, ps5_):
                    # L5 (512->3) k-group ki: accumulate into one shared
                    # PSUM bank, each sub-tile at col offset 32*s
                    for s in range(SUB):
                        nc.tensor.matmul(
                            ps5_[32 * s:32 * s + 3, :],
                            w_sb[4][ki][:, 0:3], hs[s][0][0:128, :],
                            start=(ki == 0), stop=(ki == 3),
                            tile_position=(0, 32 * s))

                for layer, (K, M) in enumerate(DIMS[:4]):
                    kchunks = _chunks(K)
                    mchunks = _chunks(M)
                    nxt = []
                    if layer == 3:
                        ps5 = psum.tile([99, BT], f32, name="ps5", tag="ps")
                    for mi, (ms, mc) in enumerate(mchunks):
                        # where this output chunk must live for the consumer
                        packs_next = (layer < 4 and mc <= 64)
                        pss = [None] * SUB
                        obases = []
                        for s in range(SUB):
                            ob = 64 if (packs_next and s % 2 == 1) else 0
                            obases.append(ob)
                        for ki, (ks, kc) in enumerate(kchunks):
                            for s in range(SUB):
                                if pss[s] is None:
                                    # allocate at first use: spreads the
                                    # slot-wait out over the k0 sweep
                                    pss[s] = psum.tile([128, BT], f32,
                                                       name="ps", tag="ps")
                                h_t, ib = cur[ki][s]
                                nc.tensor.matmul(
                                    pss[s][obases[s]:obases[s] + mc, :],
                                    w_sb[layer][ki][ib:ib + kc, ms:ms + mc],
                                    h_t[ib:ib + kc, :],
                                    start=(ki == 0),
                                    stop=(ki == len(kchunks) - 1),
                                    tile_position=(ib, obases[s]))
                        if layer == 3 and mi >= 1:
                            _l5_mms(mi - 1, nxt[mi - 1], ps5)
                        hs = []
                        for s in range(SUB):
                            ob = obases[s]
                            h = acts.tile([128, BT], store_dt, name="h",
                                          tag="h")
                            evict(h[ob:ob + mc, :],
                                  pss[s][ob:ob + mc, :], layer, mi)
                            hs.append((h, ob))
                        nxt.append(hs)
                    cur = nxt

                _l5_mms(3, cur[3], ps5)
                for s in range(SUB):
                    evict(t5[:, s * BT:(s + 1) * BT],
                          ps5[32 * s:32 * s + 3, :], 4, 0)
                if use_scale:
                    t5s = outs.tile([3, SUB * BT], f32, tag="t5s")
                    nc.vector.tensor_scalar(
                        out=t5s, in0=t5, scalar1=asc_sb, scalar2=abi_sb,
                        op0=mybir.AluOpType.mult, op1=mybir.AluOpType.add)
                    t5 = t5s
                # one store per super-tile, transposed+permuted; the host
                # undoes the column permutation during unshard
                nc.sync.dma_start(
                    out=out_d[:, ts0 * BT:(ts0 + SUB) * BT], in_=t5)

    nc.compile()
    return nc


_CACHE = {}


def _np_store_dtype():
    if COMPUTE == "f32r":
        return np.float32
    if COMPUTE == "fp16":
        return np.float16
    import ml_dtypes
    return ml_dtypes.bfloat16


def _get_nc(use_bias, use_scale):
    key = (use_bias, use_scale, COMPUTE, BT, SUB)
    if key not in _CACHE:
        _CACHE[key] = build_nc(use_bias, use_scale)
    return _CACHE[key]


def prepare(inputs):
    """Fold weights and build per-core input maps. Returns (nc, in_maps)."""
    fw = fold_weights(inputs)
    use_bias = any(np.any(bi != 0.0) for bi in fw['b'])
    use_scale = bool(np.any(fw['asc'] != 1.0) or np.any(fw['abi'] != 0.0))
    nc = _get_nc(use_bias, use_scale)

    sdt = _np_store_dtype()
    wcast = [np.ascontiguousarray(w.astype(sdt)) for w in fw['w']]
    xdt = np.float32 if COMPUTE == "f32r" else sdt
    x = np.ascontiguousarray(np.asarray(inputs['x'], np.float32)
                             .reshape(B, N).astype(xdt))

    ident_np = np.eye(128, dtype=xdt)
    in_maps = []
    for c in range(N_CORES):
        m = {"x": np.ascontiguousarray(x[c * B_CORE:(c + 1) * B_CORE]),
             "ident": ident_np}
        for i in range(5):
            m[f"w{i}"] = wcast[i]
        if use_bias:
            for i in range(5):
                m[f"b{i}"] = fw['b'][i].astype(np.float32)
        if use_scale:
            m["asc"] = fw['asc'].astype(np.float32)
            m["abi"] = fw['abi'].astype(np.float32)
        in_maps.append(m)
    return nc, in_maps


def gather(results):
    """Unshard: per-core out is [3, B_CORE] with column (t, j_hi, p) ->
    batch p*R + t*J + j_hi. Undo the permutation and stack cores."""
    J = BT // 128
    parts = []
    for c in range(N_CORES):
        o = np.asarray(results[c]["out"])          # [3, B_CORE]
        o = o.reshape(3, N_TILES, J, 128).transpose(3, 1, 2, 0)
        parts.append(o.reshape(B_CORE, 3))
    return np.ascontiguousarray(np.concatenate(parts, axis=0)
                                .astype(np.float32))


def kernel(**inputs):
    from concourse.bass_utils import run_bass_kernel_spmd

    nc, in_maps = prepare(inputs)
    res = run_bass_kernel_spmd(nc, in_maps, core_ids=list(range(N_CORES)))
    return gather(res.results)
